# revision 1
# baseline (speedup 1.0000x reference)
"""Trainium2 Bass kernel for the LSTM caption decoder (nn_Decoder_62483184222858).

Math (per reference):
    emb = embed_W[captions]                      # [B, T, E]
    h0 = feature, c0 = 0
    for t in 0..T-2:
        gates = x_t @ W_ih.T + h @ W_hh.T + (b_ih + b_hh)   # [B, 4H] order i,f,g,o
        i, f, o = sigmoid(...); g = tanh(g)
        c = f*c + i*g
        h = o*tanh(c) + feature                   # emitted output AND carried state
    logits = outs @ lin_W.T + lin_b               # [B, T-1, V]

Strategy: data-parallel over 8 NeuronCores (64 batch rows each).
Device-side:
  phase A: token_proj[v] = embed_W[v] @ W_ih.T + (b_ih+b_hh)  -> DRAM [V, 4H]
  loop:    x_proj gathered by token id (indirect DMA);
           PE streams h @ W_hh.T into PSUM per gate; DVE adds x_proj;
           ACT sigmoid/tanh; DVE c/h updates; PE transposes h -> hT
           (stationary operand of next step); hT stashed to DRAM (bf16).
           The o-gate + c/h tail is processed in two hidden-halves so the
           first hT chunks are ready ~1.5us after the last gate matmul.
  phase C: logits = outsT.T @ lin_W.T + lin_b at M=128, written per 2 steps.

NOTE: TRN2 engine instructions support ONE semaphore wait each; the Bacc
layer (nc.compile()) legalizes multi-wait joins via InstEventSemaphore.
Build with bacc.Bacc, not raw bass.Bass, or walrus rejects the program
("Too many sync wait commands").
"""

import sys

if "/opt/trn_rl_repo" not in sys.path:
    sys.path.insert(0, "/opt/trn_rl_repo")

import numpy as np
import ml_dtypes

import concourse.bass as bass
import concourse.mybir as mybir
import concourse.tile as tile
from concourse import bacc
from concourse.bass_utils import run_bass_kernel_spmd
from concourse.masks import make_identity

F32 = mybir.dt.float32
BF16 = mybir.dt.bfloat16
I32 = mybir.dt.int32
AF = mybir.ActivationFunctionType

EMBED, HIDDEN, VOCAB = 512, 1024, 1004
B, T = 512, 65
NCORES = 8
BL = B // NCORES          # 64 batch rows per core
TS = T - 1                # 64 time steps
G4 = 4 * HIDDEN           # 4096 gate width
KK_H = HIDDEN // 128      # 8 contraction chunks over hidden
KK_E = EMBED // 128       # 4 contraction chunks over embed
NVT = (VOCAB + 127) // 128  # 8 vocab tiles (last is 108 rows)
HH = HIDDEN // 2          # 512: half-hidden tail granularity

# blob_a (bf16) layout: embWT | WihT
A_EMB = 0
A_WIH = A_EMB + KK_E * VOCAB            # 4016
A_END = A_WIH + KK_E * G4               # 20400
# blob_b (bf16) layout: featT | W_hh gate-major blocks
# W block bi = (gate_stream_idx*2 + half)*KK_H + k, 512 cols each,
# gate stream order [g, i, f, o]
B_FT = 0
B_WHH = KK_H * BL                       # 512
B_END = B_WHH + KK_H * G4               # 33280


def build_program(steps=TS):
    nc = bacc.Bacc("TRN2", target_bir_lowering=False, debug=False)

    blob_a = nc.dram_tensor("blob_a", [128, A_END], BF16, kind="ExternalInput")
    blob_b = nc.dram_tensor("blob_b", [128, B_END], BF16, kind="ExternalInput")
    biasg = nc.dram_tensor("biasg", [1, G4], F32, kind="ExternalInput")
    feat = nc.dram_tensor("feat", [BL, HIDDEN], BF16, kind="ExternalInput")
    caps = nc.dram_tensor("caps", [BL, TS], I32, kind="ExternalInput")
    linWT = nc.dram_tensor("linWT", [128, KK_H * VOCAB], BF16, kind="ExternalInput")
    linb = nc.dram_tensor("linb", [1, VOCAB], F32, kind="ExternalInput")
    out = nc.dram_tensor("out", [BL, TS, VOCAB], F32, kind="ExternalOutput")

    tokp = nc.dram_tensor("tokp", [VOCAB, G4], F32, kind="Internal")
    outsT = nc.dram_tensor("outsT", [KK_H, 128, TS * BL], BF16, kind="Internal")

    with tile.TileContext(nc) as tc:
        _body(nc, tc, steps,
              blob_a.ap(), blob_b.ap(), biasg.ap(), feat.ap(), caps.ap(),
              linWT.ap(), linb.ap(), out.ap(), tokp.ap(), outsT.ap())
    nc.compile()
    return nc


def _body(nc, tc, steps, blob_a, blob_b, biasg, feat, caps, linWT, linb, out,
          tokp, outsT):
    with (
        tc.tile_pool(name="pg", bufs=1) as pg,
        tc.tile_pool(name="pgp", bufs=1, space="PSUM") as pgp,
    ):
        ident = pg.tile([BL, BL], BF16, tag="ident")
        make_identity(nc, ident[:])

        # ================= phase A: token_proj ============================
        with (
            tc.tile_pool(name="pa", bufs=1) as pa,
            tc.tile_pool(name="pap", bufs=1, space="PSUM") as pap,
        ):
            ba = pa.tile([128, A_END], BF16, tag="blob_a")
            nc.sync.dma_start(ba[:], blob_a)
            embWT_sb = ba[:, A_EMB:A_EMB + KK_E * VOCAB]
            WihT_sb = ba[:, A_WIH:A_WIH + KK_E * G4]
            bias_sb = pa.tile([128, G4], F32, tag="bias")
            nc.sync.dma_start(bias_sb[:], biasg.to_broadcast((128, G4)))

            # prefetch later-phase constants (behind phase A's own loads on
            # the same HWDGE ring, so phase A starts ASAP)
            bb = pg.tile([128, B_END], BF16, tag="blob_b")
            nc.sync.dma_start(bb[:, B_FT:B_FT + KK_H * BL],
                              blob_b[:, B_FT:B_FT + KK_H * BL])
            GW = 2 * KK_H * 512  # cols per gate (2 halves x 8 k-chunks)
            for gi in range(4):
                nc.sync.dma_start(
                    bb[:, B_WHH + gi * GW:B_WHH + (gi + 1) * GW],
                    blob_b[:, B_WHH + gi * GW:B_WHH + (gi + 1) * GW])
            W_sb = bb[:, B_WHH:B_WHH + KK_H * G4]
            hT_init = bb[:, B_FT:B_FT + KK_H * BL]
            feat_sb = pg.tile([BL, HIDDEN], BF16, tag="feat")
            nc.sync.dma_start(feat_sb[:], feat)
            caps_sb = pg.tile([BL, TS], I32, tag="caps")
            nc.sync.dma_start(caps_sb[:], caps)
            linWT_sb = pg.tile([128, KK_H * VOCAB], BF16, tag="linWT")
            nc.sync.dma_start(linWT_sb[:], linWT)
            lb_sb = pg.tile([128, VOCAB], F32, tag="lb")
            nc.sync.dma_start(lb_sb[:], linb.to_broadcast((128, VOCAB)))

            for m in range(NVT):
                mrows = min(128, VOCAB - 128 * m)
                for nb in range(G4 // 512):
                    ps = pap.tile([128, 512], F32, tag="tp_ps", bufs=8)
                    for k in range(KK_E):
                        nc.tensor.matmul(
                            ps[:mrows],
                            lhsT=embWT_sb[:, k * VOCAB + 128 * m:
                                          k * VOCAB + 128 * m + mrows],
                            rhs=WihT_sb[:, k * G4 + 512 * nb:
                                        k * G4 + 512 * (nb + 1)],
                            start=(k == 0), stop=(k == KK_E - 1),
                        )
                    sb = pa.tile([128, 512], F32, tag="tp_sb", bufs=8)
                    nc.vector.tensor_add(
                        sb[:mrows], ps[:mrows],
                        bias_sb[:mrows, 512 * nb:512 * (nb + 1)])
                    nc.sync.dma_start(
                        tokp[128 * m:128 * m + mrows, 512 * nb:512 * (nb + 1)],
                        sb[:mrows])

        # ================= phase B: recurrence + in-loop logits ===========
        with (
            tc.tile_pool(name="pb", bufs=1) as pb,
            tc.tile_pool(name="pbp", bufs=1, space="PSUM") as pbp,
        ):
            c_cur = [None, None]
            for hh in range(2):
                c_cur[hh] = pb.tile([BL, HH], F32, tag=f"c{hh}", bufs=2,
                                    name=f"c0_{hh}")
                nc.vector.memset(c_cur[hh][:], 0.0)
            # hT halves: [128, 4*BL] each, kk 0..3 in half 0, 4..7 in half 1
            hT_cur = [hT_init[:, 0:4 * BL], hT_init[:, 4 * BL:8 * BL]]

            lpend = {}

            def logits_half(p, nh):
                # logits for steps 2p, 2p+1 (M=128 tokens) from stashed
                # outsT; the two vocab halves are independent accumulation
                # groups issued on consecutive steps as PE gap filler.
                if nh == 0:
                    lt = pb.tile([128, KK_H * 128], BF16, tag="lhsT", bufs=3,
                                 name=f"lt_{p}")
                    nc.sync.dma_start(
                        lt[:].rearrange("p (k b) -> p k b", k=KK_H),
                        outsT[:, :, 128 * p:128 * (p + 1)].rearrange(
                            "k p b -> p k b"))
                    lp = pbp.tile([128, 1024], F32, tag="l_ps", bufs=1,
                                  name=f"lp_{p}")
                    lpend[p] = (lt, lp)
                lt, lp = lpend[p]
                n0, n1 = (0, 512) if nh == 0 else (512, VOCAB)
                for k in range(KK_H):
                    nc.tensor.matmul(
                        lp[:, n0:n1],
                        lhsT=lt[:, 128 * k:128 * (k + 1)],
                        rhs=linWT_sb[:, k * VOCAB + n0:k * VOCAB + n1],
                        start=(k == 0), stop=(k == KK_H - 1))
                if nh == 1:
                    del lpend[p]
                    ls = pb.tile([128, VOCAB], F32, tag="ls", bufs=2,
                                 name=f"ls_{p}")
                    nc.vector.tensor_add(ls[:], lp[:, 0:VOCAB], lb_sb[:])
                    nc.sync.dma_start(out[:, 2 * p, :], ls[0:BL])
                    nc.sync.dma_start(out[:, 2 * p + 1, :], ls[BL:128])

            def logits_pair(p):
                logits_half(p, 0)
                logits_half(p, 1)

            # stream order: g, i, f, o  (o last; g early so the c-chain
            # completes while o streams; o feeds the critical h tail).
            # colg = column base in torch gate order (for xp slicing).
            GATES = [(2048, AF.Tanh), (0, AF.Sigmoid), (1024, AF.Sigmoid),
                     (3072, AF.Sigmoid)]

            for t in range(steps):
                xp = pb.tile([BL, G4], F32, tag="xp", bufs=2)
                nc.gpsimd.indirect_dma_start(
                    out=xp[:], out_offset=None, in_=tokp,
                    in_offset=bass.IndirectOffsetOnAxis(
                        ap=caps_sb[:, t:t + 1], axis=0),
                )

                def mm_gate(psum_ap, gi, hh, k):
                    half, off = divmod(k, 4)
                    bi = (gi * 2 + hh) * KK_H + k
                    nc.tensor.matmul(
                        psum_ap,
                        lhsT=hT_cur[half][:, off * BL:(off + 1) * BL],
                        rhs=W_sb[:, bi * 512:(bi + 1) * 512],
                        start=(k == 0), stop=(k == KK_H - 1),
                    )

                # gates g,i,f: half-chunks [BL, 512]; o-gate: quarter
                # chunks [BL, 256] so its tail chain starts ~2.5us before
                # the stream ends.  One shared 1-bank psum tag for all.
                act = {}
                for gi, (colg, fn) in enumerate(GATES):
                    if gi == 3:
                        break
                    for hh in range(2):
                        gp = pbp.tile([BL, HH], F32, tag="g_ps", bufs=4,
                                      name=f"gp{gi}{hh}_{t}")
                        for k in range(KK_H):
                            mm_gate(gp[:], gi, hh, k)
                        gs = pb.tile([BL, HH], F32, tag=f"gs{gi}{hh}", bufs=1,
                                     name=f"gs{gi}{hh}_{t}")
                        nc.vector.tensor_add(
                            gs[:], gp[:],
                            xp[:, colg + HH * hh:colg + HH * (hh + 1)])
                        a = pb.tile([BL, HH], F32, tag=f"a{gi}{hh}", bufs=1,
                                    name=f"a{gi}{hh}_{t}")
                        nc.scalar.activation(a[:], gs[:], fn)
                        act[(gi, hh)] = a
                    if gi == 2:
                        # c-chain (needs g,i,f) runs while the o-gate streams
                        c_new, tc_h = [None, None], [None, None]
                        for hh in range(2):
                            t1 = pb.tile([BL, HH], F32, tag=f"t1{hh}", bufs=1,
                                         name=f"t1{hh}_{t}")
                            nc.vector.tensor_mul(
                                t1[:], act[(2, hh)][:], c_cur[hh][:])
                            t2 = pb.tile([BL, HH], F32, tag=f"t2{hh}", bufs=1,
                                         name=f"t2{hh}_{t}")
                            nc.vector.tensor_mul(
                                t2[:], act[(1, hh)][:], act[(0, hh)][:])
                            c_new[hh] = pb.tile([BL, HH], F32, tag=f"c{hh}",
                                                bufs=2, name=f"cn{hh}_{t}")
                            nc.vector.tensor_add(c_new[hh][:], t1[:], t2[:])
                            tc_h[hh] = pb.tile([BL, HH], BF16, tag=f"tc{hh}",
                                               bufs=1, name=f"tch{hh}_{t}")
                            nc.scalar.activation(
                                tc_h[hh][:], c_new[hh][:], AF.Tanh)

                # o-gate quarters + h tail: t3 = o'*tanh(c);
                # h = t3 + feature; transpose -> hT half; stash to DRAM
                QQ = HH // 2  # 256
                hT_new = [None, None]
                for hh in range(2):
                    sl = slice(HH * hh, HH * (hh + 1))
                    h = pb.tile([BL, HH], BF16, tag=f"h{hh}", bufs=2,
                                name=f"h{hh}_{t}")
                    for q in range(2):
                        qq = 2 * hh + q
                        gq = pbp.tile([BL, QQ], F32, tag="g_ps", bufs=4,
                                      name=f"gq{qq}_{t}")
                        for k in range(KK_H):
                            half, off = divmod(k, 4)
                            bi = (3 * 2 + hh) * KK_H + k
                            nc.tensor.matmul(
                                gq[:],
                                lhsT=hT_cur[half][:, off * BL:(off + 1) * BL],
                                rhs=W_sb[:, bi * 512 + QQ * q:
                                         bi * 512 + QQ * (q + 1)],
                                start=(k == 0), stop=(k == KK_H - 1),
                            )
                        go = pb.tile([BL, QQ], F32, tag=f"go{qq}", bufs=1,
                                     name=f"go{qq}_{t}")
                        nc.vector.tensor_add(
                            go[:], gq[:],
                            xp[:, 3072 + QQ * qq:3072 + QQ * (qq + 1)])
                        oa = pb.tile([BL, QQ], BF16, tag=f"oa{qq}", bufs=1,
                                     name=f"oa{qq}_{t}")
                        nc.scalar.activation(oa[:], go[:], AF.Sigmoid)
                        t3 = pb.tile([BL, QQ], BF16, tag=f"t3{qq}", bufs=1,
                                     name=f"t3{qq}_{t}")
                        nc.vector.tensor_mul(
                            t3[:], oa[:], tc_h[hh][:, QQ * q:QQ * (q + 1)])
                        nc.vector.tensor_add(
                            h[:, QQ * q:QQ * (q + 1)], t3[:],
                            feat_sb[:, HH * hh + QQ * q:
                                    HH * hh + QQ * (q + 1)])
                    hp = pbp.tile([128, 4 * BL], BF16, tag=f"h_ps{hh}", bufs=1,
                                  name=f"hp{hh}_{t}")
                    for k4 in range(4):
                        nc.tensor.matmul(
                            hp[:, k4 * BL:(k4 + 1) * BL],
                            lhsT=h[:, 128 * k4:128 * (k4 + 1)],
                            rhs=ident[:],
                            is_transpose=True,
                            start=(k4 == 0), stop=(k4 == 3),
                        )
                    hT_new[hh] = pb.tile([128, 4 * BL], BF16,
                                         tag=f"hT{hh}", bufs=2,
                                         name=f"hTn{hh}_{t}")
                    nc.vector.tensor_copy(hT_new[hh][:], hp[:])
                    nc.sync.dma_start(
                        outsT[4 * hh:4 * (hh + 1), :,
                              t * BL:(t + 1) * BL].rearrange("k p b -> p k b"),
                        hT_new[hh][:].rearrange("p (k b) -> p k b", k=4))

                hT_cur = [hT_new[0][:], hT_new[1][:]]
                c_cur = c_new

                # fill the h-tail PE gap with logits for an old step pair
                if t >= 4 and t % 2 == 0:
                    logits_pair(t // 2 - 2)

            # remaining logits pairs
            for p in range(max(0, steps // 2 - 2), (steps + 1) // 2):
                logits_pair(p)

# ---------------------------------------------------------------------------
# host glue
# ---------------------------------------------------------------------------

_CACHE = {}


def _get_program(steps=TS):
    if steps not in _CACHE:
        _CACHE[steps] = build_program(steps)
    return _CACHE[steps]


def make_in_maps(feature, captions, embed_W, W_ih, W_hh, b_ih, b_hh,
                 lin_W, lin_b):
    f32 = np.float32
    bf16 = ml_dtypes.bfloat16

    def chunkT(w, kk):
        # [R, C] -> transpose -> [kk, 128, C] -> [128, kk*C] (per-partition
        # free-dim layout: chunk-major)
        wt = np.ascontiguousarray(w.T.astype(f32))
        r = wt.reshape(kk, 128, w.shape[0])
        return np.ascontiguousarray(r.transpose(1, 0, 2).reshape(128, -1))

    embWT_p = chunkT(embed_W, KK_E)          # [128, 4*1004]
    WihT_p = chunkT(W_ih, KK_E)              # [128, 4*4096]
    blob_a = np.concatenate([embWT_p, WihT_p], axis=1).astype(bf16)

    # W_hh gate-major: block bi=(gi*2+hh)*8+k holds W_hh.T[k-chunk,
    # gate_src_col + 512*hh : +512]; stream gate order [g, i, f, o]
    wt = np.ascontiguousarray(W_hh.T.astype(f32)).reshape(KK_H, 128, G4)
    blocks = []
    for src_colg in (2048, 0, 1024, 3072):
        for hh in range(2):
            for k in range(KK_H):
                blocks.append(wt[k, :, src_colg + 512 * hh:
                                 src_colg + 512 * (hh + 1)])
    WhhT_p = np.concatenate(blocks, axis=1)       # [128, 8*4096]
    linWT_p = chunkT(lin_W, KK_H).astype(bf16)   # [128, 8*1004]

    shared = {
        "blob_a": np.ascontiguousarray(blob_a),
        "biasg": (b_ih + b_hh).astype(f32).reshape(1, G4),
        "linWT": np.ascontiguousarray(linWT_p),
        "linb": lin_b.astype(f32).reshape(1, VOCAB),
    }
    in_maps = []
    for i in range(NCORES):
        sl = slice(i * BL, (i + 1) * BL)
        fl = np.ascontiguousarray(feature[sl].astype(f32))
        featT_p = np.ascontiguousarray(
            fl.T.reshape(KK_H, 128, BL).transpose(1, 0, 2).reshape(128, -1))
        blob_b = np.concatenate([featT_p, WhhT_p], axis=1).astype(bf16)
        m = dict(shared)
        m["blob_b"] = np.ascontiguousarray(blob_b)
        m["feat"] = fl.astype(bf16)
        m["caps"] = np.ascontiguousarray(captions[sl, :TS].astype(np.int32))
        in_maps.append(m)
    return in_maps


def kernel(feature, captions, lengths=None, embed_W=None, W_ih=None,
           W_hh=None, b_ih=None, b_hh=None, lin_W=None, lin_b=None,
           trace=False):
    feature = np.asarray(feature)
    captions = np.asarray(captions)
    nc = _get_program()
    in_maps = make_in_maps(
        feature, captions, np.asarray(embed_W), np.asarray(W_ih),
        np.asarray(W_hh), np.asarray(b_ih), np.asarray(b_hh),
        np.asarray(lin_W), np.asarray(lin_b))
    res = run_bass_kernel_spmd(nc, in_maps, list(range(NCORES)), trace=trace)
    outp = np.concatenate([res.results[i]["out"] for i in range(NCORES)], axis=0)
    if trace:
        kernel.last_exec_time_ns = res.exec_time_ns
        kernel.last_results = res
    return outp



# revision 2
# speedup vs baseline: 2.7613x; 2.7613x over previous
"""Trainium2 Bass kernel v2 for the LSTM caption decoder.

Architecture (vs v1 baseline): the gate matmuls are FLIPPED — weights are the
stationary operand [K=128, M=128 gate units] and the hidden state streams as
the moving operand in transposed layout [K, batch=64].  TimelineSim charges
out_free x cycles_per_row, so the per-step gate cost drops from 32768 cycles
(h-stationary, M=64 wastes half the array) to 16384 in bf16 — and to 6144
with fp8e4 DoubleRow (2 K-tiles per instruction at 0.5 cyc/row).

Consequences of the flip:
  - gates emerge in [gate_unit, batch] = transposed layout; the whole
    elementwise c/h chain runs in [hidden, batch]; h_new IS the next step's
    moving operand. No per-step PE transposes of h.
  - the x-projection is FUSED into the gate matmul as 4 extra contraction
    chunks (W_ih columns): no token_proj phase, no [64,4096] gather; instead a
    small per-step embedding row gather [64,512] + 4 PE transposes.
  - logits keep h-stationary orientation (lhsT = h pair [128, 2x64],
    moving = lin_W.T in bf16) and run as PE gap filler, one vocab half per
    step. fp8 is NOT used for logits (accuracy).

Scaling for fp8e4 (ml_dtypes.float8_e4m3, max 240): W x 1024, x/h x 16;
psum is scaled 2^14, descaled in the ACT that applies the nonlinearity.
Numerics sim: rel err 9.0e-3 vs tolerance 2e-2.

Bias note: b_ih+b_hh from setup_inputs is always zero. If nonzero, an extra
stationary block + constant moving column adds the bias per gate unit.
"""

import sys

if "/opt/trn_rl_repo" not in sys.path:
    sys.path.insert(0, "/opt/trn_rl_repo")

import numpy as np
import ml_dtypes

import concourse.bass as bass
import concourse.mybir as mybir
import concourse.tile as tile
from concourse import bacc
from concourse.bass_utils import run_bass_kernel_spmd
from concourse.masks import make_identity

F32 = mybir.dt.float32
BF16 = mybir.dt.bfloat16
F8 = mybir.dt.float8e4
I32 = mybir.dt.int32
AF = mybir.ActivationFunctionType
ALU = mybir.AluOpType
DR = mybir.MatmulPerfMode.DoubleRow
NPF8 = ml_dtypes.float8_e4m3
NPBF = ml_dtypes.bfloat16

EMBED, HIDDEN, VOCAB = 512, 1024, 1004
B, T = 512, 65
NCORES = 8
BL = B // NCORES          # 64 batch rows per core
TS = T - 1                # 64 time steps
G4 = 4 * HIDDEN           # 4096 gate width
NM = G4 // 128            # 32 stationary m-tiles
NKI = EMBED // 128        # 4 ih k-chunks -> 2 dk pairs
NKH = HIDDEN // 128       # 8 hh k-chunks -> 4 dk pairs
NDK = (NKI + NKH) // 2    # 6 dk pairs per m-tile
VH = VOCAB // 2           # 502 vocab half

SW = 1024.0               # weight scale for fp8
SX = 16.0                 # x/h scale for fp8
DESCALE = 1.0 / (SW * SX)

# m-tile issue order: gate stream order g, i, o, f (torch gate index 2,0,3,1)
# f is streamed LAST: it heads the serial c->tanh->h chain, and putting it
# last lets the next step's ih matmuls for g/i/o start with no psum WAR wait.
GSTREAM = [2, 0, 3, 1]
MORDER = [g * 8 + j for g in GSTREAM for j in range(8)]

WCOLS = NM * NDK * 256    # 49152 fp8 cols of the stationary blob


def build_program(steps=TS, with_bias=False, with_linb=False):
    nc = bacc.Bacc("TRN2", target_bir_lowering=False, debug=False)

    wblob = nc.dram_tensor("wblob", [128, WCOLS], F8, kind="ExternalInput")
    embbf = nc.dram_tensor("embbf", [VOCAB, EMBED], BF16, kind="ExternalInput")
    linwt = nc.dram_tensor("linwt", [128, NKH * VOCAB], BF16,
                           kind="ExternalInput")
    linb = nc.dram_tensor("linb", [1, VOCAB], BF16, kind="ExternalInput")
    featbf = nc.dram_tensor("featbf", [128, 512], BF16, kind="ExternalInput")
    featsx = nc.dram_tensor("featsx", [128, 512], F32, kind="ExternalInput")
    h0f8 = nc.dram_tensor("h0f8", [128, 512], F8, kind="ExternalInput")
    caps = nc.dram_tensor("caps", [BL, TS], I32, kind="ExternalInput")
    if with_bias:
        bblk = nc.dram_tensor("bblk", [128, NM * 256], F8, kind="ExternalInput")
    else:
        bblk = None
    outd = nc.dram_tensor("out", [BL, TS, VOCAB], F32, kind="ExternalOutput")

    with tile.TileContext(nc) as tc:
        _body(nc, tc, steps, with_bias, with_linb, wblob.ap(), embbf.ap(),
              linwt.ap(), linb.ap(), featbf.ap(), featsx.ap(), h0f8.ap(),
              caps.ap(), bblk.ap() if bblk is not None else None, outd.ap())
    nc.compile()
    return nc


def _body(nc, tc, steps, with_bias, with_linb, wblob, embbf, linwt, linb,
          featbf, featsx, h0f8, caps, bblk, outd):
    with (
        tc.tile_pool(name="pg", bufs=1) as pg,
        tc.tile_pool(name="pb", bufs=1) as pb,
        tc.tile_pool(name="pp", bufs=1, space="PSUM") as pp,
    ):
        # ---------------- startup loads ----------------
        # small, early-needed tensors first: caps gates the first embedding
        # gather; feat/h0 gate the first h-dependent matmuls
        caps_sb = pg.tile([BL, TS], I32, tag="cap")
        nc.sync.dma_start(caps_sb[:], caps)
        ident = pg.tile([BL, BL], BF16, tag="id")
        make_identity(nc, ident[:])
        hf8_prev = pb.tile([128, 512], F8, tag="hf8", bufs=2, name="hf8_init")
        nc.sync.dma_start(hf8_prev[:], h0f8)
        c_prev = pb.tile([128, 512], BF16, tag="c", bufs=2, name="c_init")
        nc.vector.memset(c_prev[:], 0.0)
        featbf_sb = pg.tile([128, 512], BF16, tag="fb")
        nc.sync.dma_start(featbf_sb[:], featbf)
        featsx_sb = pg.tile([128, 512], F32, tag="fs")
        nc.sync.dma_start(featsx_sb[:], featsx)
        w_sb = pg.tile([128, WCOLS], F8, tag="w")
        # weight blocks in gate processing order (f, g, i, o) so the first
        # gate groups can start before the whole blob lands
        for g in (1, 2, 0, 3):
            c0, c1 = g * 8 * 1536, (g + 1) * 8 * 1536
            nc.sync.dma_start(w_sb[:, c0:c1], wblob[:, c0:c1])
        linwt_sb = pg.tile([128, NKH * VOCAB], BF16, tag="lw")
        nc.sync.dma_start(linwt_sb[:], linwt)
        if with_linb:
            linbbf_sb = pg.tile([1, VOCAB], BF16, tag="lb")
            nc.sync.dma_start(linbbf_sb[:], linb)
            onesrow_sb = pg.tile([1, 128], BF16, tag="ones1")
            nc.vector.memset(onesrow_sb[:], 1.0)
        if with_bias:
            bb_sb = pg.tile([128, NM * 256], F8, tag="bb")
            nc.sync.dma_start(bb_sb[:], bblk)
            ones_sb = pg.tile([128, 256], F8, tag="ones")
            nc.vector.memset(ones_sb[:], 0.0)
            nc.vector.memset(ones_sb[0:1, 0:64], SX)

        # ---------------- helpers ----------------
        def gather(t):
            e = pb.tile([BL, EMBED], BF16, tag="emb", bufs=4, name=f"emb_{t}")
            nc.gpsimd.indirect_dma_start(
                out=e[:], out_offset=None, in_=embbf,
                in_offset=bass.IndirectOffsetOnAxis(
                    ap=caps_sb[:, t:t + 1], axis=0))
            return e

        def transp(t, emb_t):
            ps = pp.tile([128, 256], BF16, tag="et", bufs=2, name=f"etp_{t}")
            for cchunk in range(4):
                nc.tensor.matmul(
                    ps[:, 64 * cchunk:64 * (cchunk + 1)],
                    lhsT=emb_t[:, 128 * cchunk:128 * (cchunk + 1)],
                    rhs=ident[:], is_transpose=True)
            ef = pb.tile([128, 256], F8, tag="ef8", bufs=2, name=f"ef8_{t}")
            nc.vector.tensor_scalar_mul(ef[:], ps[:], SX)
            return ef

        def dr_mm(out_ap, wcol, rhs_ap, start, stop):
            nc.tensor.matmul(
                out_ap,
                lhsT=w_sb[:, wcol:wcol + 256].rearrange(
                    "p (two m) -> p two m", two=2),
                rhs=rhs_ap, start=start, stop=stop, perf_mode=DR,
                skip_group_check=True)

        # pre-loop: embeddings for steps 0 and 1
        emb_t = {0: gather(0)}
        if steps > 1:
            emb_t[1] = gather(1)
        ef8_t = {0: transp(0, emb_t[0])}

        hp_t = {}         # hpair tiles by pair index

        def logits_half(t):
            # pair p halves run at iterations 2p+3 / 2p+4 — both at least
            # two steps after hp(p) completes, so the PE never waits on it
            p, nh = (t - 3) // 2, (t - 3) % 2
            if p > (steps - 2) // 2:
                return
            # separate psum tile per vocab half (own bank, no false deps)
            lp = pp.tile([128, 512], F32, tag=f"lps{nh}", bufs=1,
                         name=f"lp{nh}_{p}")
            lp_pairs.setdefault(p, [None, None])[nh] = lp
            hp = hp_t[p]
            s0 = nh * VH          # vocab start in lin_W
            for k in range(NKH):
                # hp layout [128, (chunk 8, step 2, batch 64)] — chunk-major
                # so the stationary lhsT [128, 128] is contiguous
                lhsT = hp[:, 128 * k:128 * (k + 1)]
                nc.tensor.matmul(
                    lp[:, 0:VH], lhsT=lhsT,
                    rhs=linwt_sb[:, k * VOCAB + s0:k * VOCAB + s0 + VH],
                    start=(k == 0), stop=(k == NKH - 1) and not with_linb,
                    skip_group_check=True)
            if with_linb:
                # fold lin_b in as a K=1 ones-row matmul (broadcast add)
                nc.tensor.matmul(
                    lp[:, 0:VH], lhsT=onesrow_sb[:],
                    rhs=linbbf_sb[:, s0:s0 + VH],
                    start=False, stop=True, skip_group_check=True)
            if nh == 1:
                del hp_t[p]

        lp_pairs = {}   # pair -> [lp0, lp1] psum tiles awaiting copy-out

        # gate processing order: f first (it heads the serial
        # c -> tanh -> h chain), o last (needed latest by the chain)
        GORDER = [1, 2, 0, 3]   # torch gate indices f, g, i, o

        # ---------------- main loop ----------------
        for t in range(steps):
            # Pool: embedding gather two steps ahead (Pool does only DMA)
            if t + 2 < steps:
                emb_t[t + 2] = gather(t + 2)

            # PE: transposes for t+1 (DVE converts to fp8)
            if t + 1 < steps:
                ef8_t[t + 1] = transp(t + 1, emb_t.pop(t + 1))

            # per-gate psum tiles (one bank each); a gate's matmul group for
            # step t starts long after its step t-1 ACT read, so bufs=1
            # carries no WAR stall
            gt = {g: pp.tile([128, 512], F32, tag=f"gp{g}", bufs=1,
                             name=f"gp{g}_{t}") for g in GORDER}
            ef8 = ef8_t.pop(t)

            # PE: logits gap filler (one vocab half per step)
            if t >= 3:
                logits_half(t)

            # PE: gate matmul groups, gate-major, split into m-halves so
            # the ACT reads pipeline with the matmuls.  Within an m-half:
            # dk0..3 (ih + first h half) for all its m-tiles, then dk4..5.
            for g in GORDER:
                for mh in range(2):
                    for j in range(4 * mh, 4 * mh + 4):
                        m = g * 8 + j
                        reg = j * 64
                        out_ap = gt[g][:, reg:reg + 64]
                        for dk in (0, 1):
                            # exactly ONE start per psum bank per step: a
                            # start poisons the whole 2KB bank as pending-
                            # zero, which auto-zeroes every region's first
                            # write; a second start would clobber siblings
                            dr_mm(out_ap, (m * NDK + dk) * 256,
                                  ef8[:, 128 * dk:128 * (dk + 1)].rearrange(
                                      "p (two n) -> p two n", two=2),
                                  start=(mh == 0 and j == 0 and dk == 0),
                                  stop=False)
                        if with_bias:
                            nc.tensor.matmul(
                                out_ap,
                                lhsT=bb_sb[:, m * 256:m * 256 + 256]
                                .rearrange("p (two m2) -> p two m2", two=2),
                                rhs=ones_sb[:].rearrange(
                                    "p (two n) -> p two n", two=2),
                                start=False, stop=False, perf_mode=DR,
                                skip_group_check=True)
                        for dk in (2, 3):
                            dr_mm(out_ap, (m * NDK + dk) * 256,
                                  hf8_prev[:, 128 * (dk - 2):128 * (dk - 1)]
                                  .rearrange("p (two n) -> p two n", two=2),
                                  start=False, stop=False)
                    for j in range(4 * mh, 4 * mh + 4):
                        m = g * 8 + j
                        reg = j * 64
                        for dk in (4, 5):
                            dr_mm(gt[g][:, reg:reg + 64],
                                  (m * NDK + dk) * 256,
                                  hf8_prev[:, 128 * (dk - 2):128 * (dk - 1)]
                                  .rearrange("p (two n) -> p two n", two=2),
                                  start=False, stop=(dk == 5))

            # ACT: gate nonlinearities, halved, in gate stream order
            ff = pb.tile([128, 512], BF16, tag="ff", bufs=1, name=f"ff_{t}")
            gg = pb.tile([128, 512], BF16, tag="gg", bufs=1, name=f"gg_{t}")
            ii = pb.tile([128, 512], BF16, tag="ii", bufs=1, name=f"ii_{t}")
            oo = pb.tile([128, 512], BF16, tag="oo", bufs=1, name=f"oo_{t}")
            for dst, g, fn in ((ff, 1, AF.Sigmoid), (gg, 2, AF.Tanh),
                               (ii, 0, AF.Sigmoid), (oo, 3, AF.Sigmoid)):
                for q in range(2):
                    sl = slice(256 * q, 256 * (q + 1))
                    nc.scalar.activation(dst[:, sl], gt[g][:, sl], fn,
                                         scale=DESCALE)

            # Pool: t1 = f * c_prev (off the critical DVE/ACT engines)
            t1 = pb.tile([128, 512], BF16, tag="t1", bufs=1, name=f"t1_{t}")
            for q in range(2):
                sl = slice(256 * q, 256 * (q + 1))
                nc.gpsimd.tensor_mul(t1[:, sl], ff[:, sl], c_prev[:, sl])

            # DVE: t2 + c; ACT: tanh(c) — all in halves
            t2 = pb.tile([128, 512], BF16, tag="t2", bufs=1, name=f"t2_{t}")
            c_new = pb.tile([128, 512], BF16, tag="c", bufs=2, name=f"c_{t}")
            tc_h = pb.tile([128, 512], BF16, tag="tc", bufs=1, name=f"tc_{t}")
            for q in range(2):
                sl = slice(256 * q, 256 * (q + 1))
                nc.vector.tensor_mul(t2[:, sl], ii[:, sl], gg[:, sl])
                nc.vector.tensor_add(c_new[:, sl], t1[:, sl], t2[:, sl])
                nc.scalar.activation(tc_h[:, sl], c_new[:, sl], AF.Tanh)

            # tail in halves: t3 = o*tanh(c), hf8 (DVE); bf16 h for the
            # logits on Pool (SBUF-only operands)
            if t % 2 == 0:
                hp = pb.tile([128, 1024], BF16, tag="hp", bufs=3,
                             name=f"hp_{t // 2}")
                hp_t[t // 2] = hp
            else:
                hp = hp_t[t // 2]
            hf8 = pb.tile([128, 512], F8, tag="hf8", bufs=2, name=f"hf8_{t}")
            t3 = pb.tile([128, 512], BF16, tag="t3", bufs=1, name=f"t3_{t}")
            hp4 = hp[:].rearrange("p (cc s b) -> p cc s b", cc=8, s=2)
            for q in range(2):
                sl = slice(256 * q, 256 * (q + 1))
                nc.vector.tensor_mul(t3[:, sl], oo[:, sl], tc_h[:, sl])
                nc.vector.scalar_tensor_tensor(
                    out=hf8[:, sl], in0=t3[:, sl], scalar=SX,
                    in1=featsx_sb[:, sl], op0=ALU.mult, op1=ALU.add)
            for q in range(2):
                sl = slice(256 * q, 256 * (q + 1))
                nc.vector.tensor_add(
                    hp4[:, 4 * q:4 * (q + 1), t % 2, :],
                    t3[:, sl].rearrange("p (cc b) -> p cc b", cc=4),
                    featbf_sb[:, sl].rearrange("p (cc b) -> p cc b", cc=4))
            hf8_prev = hf8
            c_prev = c_new

        # ---------------- drain remaining logits ----------------
        for t in range(steps, steps + 4):
            if t >= 3:
                logits_half(t)

        # Deferred psum -> SBUF copies + output DMAs, emitted last so the
        # greedy scheduler places them only in genuinely idle ACT slots
        # (they must still land before the next-but-one pair reuses the
        # psum bank, which the scheduler's WAR handling enforces).
        for p in sorted(lp_pairs):
            lp0, lp1 = lp_pairs[p]
            ls = pb.tile([128, VOCAB], F32, tag="ls", bufs=2, name=f"ls_{p}")
            nc.scalar.activation(ls[:, 0:VH], lp0[:, 0:VH], AF.Copy)
            nc.scalar.activation(ls[:, VH:VOCAB], lp1[:, 0:VH], AF.Copy)
            nc.sync.dma_start(outd[:, 2 * p, :], ls[0:BL])
            nc.sync.dma_start(outd[:, 2 * p + 1, :], ls[BL:128])


# ---------------------------------------------------------------------------
# host glue
# ---------------------------------------------------------------------------

_CACHE = {}


def _get_program(steps=TS, with_bias=False, with_linb=False):
    key = (steps, with_bias, with_linb)
    if key not in _CACHE:
        _CACHE[key] = build_program(steps, with_bias, with_linb)
    return _CACHE[key]


def make_in_maps(feature, captions, embed_W, W_ih, W_hh, b_ih, b_hh,
                 lin_W, lin_b, steps=TS):
    f32 = np.float32
    bvec = (np.asarray(b_ih) + np.asarray(b_hh)).astype(f32)
    with_bias = bool(np.any(bvec != 0.0))
    with_linb = bool(np.any(np.asarray(lin_b) != 0.0))

    # stationary fp8 blob: block (m, dk, i) = W_all.T chunk
    W_all = np.concatenate([W_ih.astype(f32), W_hh.astype(f32)], axis=1)
    WT = np.ascontiguousarray(W_all.T) * SW           # [1536, 4096]
    arr = WT.reshape(NDK, 2, 128, NM, 128)            # [dk, i, p, m, ml]
    wblob = np.ascontiguousarray(
        arr.transpose(2, 3, 0, 1, 4).reshape(128, WCOLS)).astype(NPF8)

    linwt_p = np.ascontiguousarray(
        lin_W.astype(f32).T.reshape(NKH, 128, VOCAB)
        .transpose(1, 0, 2).reshape(128, NKH * VOCAB)).astype(NPBF)

    shared = {
        "wblob": wblob,
        "embbf": np.ascontiguousarray(embed_W.astype(f32)).astype(NPBF),
        "linwt": linwt_p,
        "linb": lin_b.astype(f32).reshape(1, VOCAB).astype(NPBF),
    }
    if with_bias:
        # block m: [p, i, ml]; only (p=0, i=0) row nonzero = b[gate]*SW
        bb = np.zeros((128, NM, 2, 128), dtype=f32)
        bb[0, :, 0, :] = (bvec * SW).reshape(NM, 128)
        shared["bblk"] = np.ascontiguousarray(
            bb.reshape(128, NM * 256)).astype(NPF8)

    in_maps = []
    for i in range(NCORES):
        sl = slice(i * BL, (i + 1) * BL)
        fl = feature[sl].astype(f32)                  # [64, 1024]
        featT = np.ascontiguousarray(
            fl.T.reshape(NKH, 128, BL).transpose(1, 0, 2).reshape(128, 512))
        m = dict(shared)
        m["featbf"] = featT.astype(NPBF)
        m["featsx"] = np.ascontiguousarray(featT * SX)
        m["h0f8"] = (featT * SX).astype(NPF8)
        cp = np.zeros((BL, TS), np.int32)
        cp[:, :steps] = captions[sl, :steps].astype(np.int32)
        m["caps"] = cp
        in_maps.append(m)
    return in_maps, with_bias, with_linb


def kernel(feature, captions, lengths=None, embed_W=None, W_ih=None,
           W_hh=None, b_ih=None, b_hh=None, lin_W=None, lin_b=None,
           trace=False, steps=TS):
    feature = np.asarray(feature)
    captions = np.asarray(captions)
    in_maps, with_bias, with_linb = make_in_maps(
        feature, captions, np.asarray(embed_W), np.asarray(W_ih),
        np.asarray(W_hh), np.asarray(b_ih), np.asarray(b_hh),
        np.asarray(lin_W), np.asarray(lin_b), steps=steps)
    nc = _get_program(steps, with_bias, with_linb)
    res = run_bass_kernel_spmd(nc, in_maps, list(range(NCORES)), trace=trace)
    outp = np.concatenate([res.results[i]["out"] for i in range(NCORES)],
                          axis=0)
    if trace:
        kernel.last_exec_time_ns = res.exec_time_ns
        kernel.last_results = res
    return outp


# revision 3
# speedup vs baseline: 2.9068x; 1.0527x over previous
"""Trainium2 Bass kernel v2 for the LSTM caption decoder.

Architecture (vs v1 baseline): the gate matmuls are FLIPPED — weights are the
stationary operand [K=128, M=128 gate units] and the hidden state streams as
the moving operand in transposed layout [K, batch=64].  TimelineSim charges
out_free x cycles_per_row, so the per-step gate cost drops from 32768 cycles
(h-stationary, M=64 wastes half the array) to 16384 in bf16 — and to 6144
with fp8e4 DoubleRow (2 K-tiles per instruction at 0.5 cyc/row).

Consequences of the flip:
  - gates emerge in [gate_unit, batch] = transposed layout; the whole
    elementwise c/h chain runs in [hidden, batch]; h_new IS the next step's
    moving operand. No per-step PE transposes of h.
  - the x-projection is FUSED into the gate matmul as 4 extra contraction
    chunks (W_ih columns): no token_proj phase, no [64,4096] gather; instead a
    small per-step embedding row gather [64,512] + 4 PE transposes.
  - logits keep h-stationary orientation (lhsT = h pair [128, 2x64],
    moving = lin_W.T in bf16) and run as PE gap filler, one vocab half per
    step. fp8 is NOT used for logits (accuracy).

Scaling for fp8e4 (ml_dtypes.float8_e4m3, max 240): W x 1024, x/h x 16;
psum is scaled 2^14, descaled in the ACT that applies the nonlinearity.
Numerics sim: rel err 9.0e-3 vs tolerance 2e-2.

Bias note: b_ih+b_hh from setup_inputs is always zero. If nonzero, an extra
stationary block + constant moving column adds the bias per gate unit.
"""

import sys

if "/opt/trn_rl_repo" not in sys.path:
    sys.path.insert(0, "/opt/trn_rl_repo")

import numpy as np
import ml_dtypes

import concourse.bass as bass
import concourse.mybir as mybir
import concourse.tile as tile
from concourse import bacc
from concourse.bass_utils import run_bass_kernel_spmd
from concourse.masks import make_identity

F32 = mybir.dt.float32
BF16 = mybir.dt.bfloat16
F8 = mybir.dt.float8e4
I32 = mybir.dt.int32
AF = mybir.ActivationFunctionType
ALU = mybir.AluOpType
DR = mybir.MatmulPerfMode.DoubleRow
NPF8 = ml_dtypes.float8_e4m3
NPBF = ml_dtypes.bfloat16

EMBED, HIDDEN, VOCAB = 512, 1024, 1004
B, T = 512, 65
NCORES = 8
BL = B // NCORES          # 64 batch rows per core
TS = T - 1                # 64 time steps
G4 = 4 * HIDDEN           # 4096 gate width
NM = G4 // 128            # 32 stationary m-tiles
NKI = EMBED // 128        # 4 ih k-chunks -> 2 dk pairs
NKH = HIDDEN // 128       # 8 hh k-chunks -> 4 dk pairs
NDK = (NKI + NKH) // 2    # 6 dk pairs per m-tile
VH = VOCAB // 2           # 502 vocab half

SW = 1024.0               # weight scale for fp8
SX = 16.0                 # x/h scale for fp8
DESCALE = 1.0 / (SW * SX)

# m-tile issue order: gate stream order g, i, o, f (torch gate index 2,0,3,1)
# f is streamed LAST: it heads the serial c->tanh->h chain, and putting it
# last lets the next step's ih matmuls for g/i/o start with no psum WAR wait.
GSTREAM = [2, 0, 3, 1]
MORDER = [g * 8 + j for g in GSTREAM for j in range(8)]

WCOLS = NM * NDK * 256    # 49152 fp8 cols of the stationary blob


def build_program(steps=TS, with_bias=False, with_linb=False):
    nc = bacc.Bacc("TRN2", target_bir_lowering=False, debug=False)

    wblob = nc.dram_tensor("wblob", [128, WCOLS], F8, kind="ExternalInput")
    embbf = nc.dram_tensor("embbf", [VOCAB, EMBED], BF16, kind="ExternalInput")
    linwt = nc.dram_tensor("linwt", [128, NKH * VOCAB], BF16,
                           kind="ExternalInput")
    linb = nc.dram_tensor("linb", [1, VOCAB], BF16, kind="ExternalInput")
    featbf = nc.dram_tensor("featbf", [128, 512], BF16, kind="ExternalInput")
    featsx = nc.dram_tensor("featsx", [128, 512], F32, kind="ExternalInput")
    h0f8 = nc.dram_tensor("h0f8", [128, 512], F8, kind="ExternalInput")
    caps = nc.dram_tensor("caps", [BL, TS], I32, kind="ExternalInput")
    if with_bias:
        bblk = nc.dram_tensor("bblk", [128, NM * 256], F8, kind="ExternalInput")
    else:
        bblk = None
    outd = nc.dram_tensor("out", [BL, TS, VOCAB], F32, kind="ExternalOutput")

    with tile.TileContext(nc) as tc:
        _body(nc, tc, steps, with_bias, with_linb, wblob.ap(), embbf.ap(),
              linwt.ap(), linb.ap(), featbf.ap(), featsx.ap(), h0f8.ap(),
              caps.ap(), bblk.ap() if bblk is not None else None, outd.ap())
    nc.compile()
    return nc


def _body(nc, tc, steps, with_bias, with_linb, wblob, embbf, linwt, linb,
          featbf, featsx, h0f8, caps, bblk, outd):
    with (
        tc.tile_pool(name="pg", bufs=1) as pg,
        tc.tile_pool(name="pb", bufs=1) as pb,
        tc.tile_pool(name="pp", bufs=1, space="PSUM") as pp,
    ):
        # ---------------- startup loads ----------------
        # small, early-needed tensors first: caps gates the first embedding
        # gather; feat/h0 gate the first h-dependent matmuls
        caps_sb = pg.tile([BL, TS], I32, tag="cap")
        nc.sync.dma_start(caps_sb[:], caps)
        ident = pg.tile([BL, BL], BF16, tag="id")
        make_identity(nc, ident[:])
        hf8_prev = pb.tile([128, 512], F8, tag="hf8", bufs=2, name="hf8_init")
        nc.sync.dma_start(hf8_prev[:], h0f8)
        c_prev = pb.tile([128, 512], BF16, tag="c", bufs=2, name="c_init")
        nc.vector.memset(c_prev[:], 0.0)
        featbf_sb = pg.tile([128, 512], BF16, tag="fb")
        nc.sync.dma_start(featbf_sb[:], featbf)
        featsx_sb = pg.tile([128, 512], F32, tag="fs")
        nc.sync.dma_start(featsx_sb[:], featsx)
        w_sb = pg.tile([128, WCOLS], F8, tag="w")
        # weight blocks in gate processing order (f, g, i, o) so the first
        # gate groups can start before the whole blob lands
        for g in (1, 2, 0, 3):
            c0, c1 = g * 8 * 1536, (g + 1) * 8 * 1536
            nc.sync.dma_start(w_sb[:, c0:c1], wblob[:, c0:c1])
        linwt_sb = pg.tile([128, NKH * VOCAB], BF16, tag="lw")
        nc.sync.dma_start(linwt_sb[:], linwt)
        if with_linb:
            linbbf_sb = pg.tile([1, VOCAB], BF16, tag="lb")
            nc.sync.dma_start(linbbf_sb[:], linb)
            onesrow_sb = pg.tile([1, 128], BF16, tag="ones1")
            nc.vector.memset(onesrow_sb[:], 1.0)
        if with_bias:
            bb_sb = pg.tile([128, NM * 256], F8, tag="bb")
            nc.sync.dma_start(bb_sb[:], bblk)
            ones_sb = pg.tile([128, 256], F8, tag="ones")
            nc.vector.memset(ones_sb[:], 0.0)
            nc.vector.memset(ones_sb[0:1, 0:64], SX)

        # ---------------- helpers ----------------
        def gather(t):
            e = pb.tile([BL, EMBED], BF16, tag="emb", bufs=4, name=f"emb_{t}")
            nc.gpsimd.indirect_dma_start(
                out=e[:], out_offset=None, in_=embbf,
                in_offset=bass.IndirectOffsetOnAxis(
                    ap=caps_sb[:, t:t + 1], axis=0))
            return e

        def transp(t, emb_t):
            ps = pp.tile([128, 256], BF16, tag="et", bufs=2, name=f"etp_{t}")
            for cchunk in range(4):
                nc.tensor.matmul(
                    ps[:, 64 * cchunk:64 * (cchunk + 1)],
                    lhsT=emb_t[:, 128 * cchunk:128 * (cchunk + 1)],
                    rhs=ident[:], is_transpose=True)
            ef = pb.tile([128, 256], F8, tag="ef8", bufs=2, name=f"ef8_{t}")
            nc.vector.tensor_scalar_mul(ef[:], ps[:], SX)
            return ef

        def dr_mm(out_ap, wcol, rhs_ap, start, stop):
            nc.tensor.matmul(
                out_ap,
                lhsT=w_sb[:, wcol:wcol + 256].rearrange(
                    "p (two m) -> p two m", two=2),
                rhs=rhs_ap, start=start, stop=stop, perf_mode=DR,
                skip_group_check=True)

        # pre-loop: embeddings for steps 0 and 1
        emb_t = {0: gather(0)}
        if steps > 1:
            emb_t[1] = gather(1)
        ef8_t = {0: transp(0, emb_t[0])}

        hp_t = {}         # hpair tiles by pair index

        def logits_half(t):
            # pair p halves run at iterations 2p+3 / 2p+4 — both at least
            # two steps after hp(p) completes, so the PE never waits on it
            p, nh = (t - 3) // 2, (t - 3) % 2
            if p > (steps - 2) // 2:
                return
            # separate psum tile per vocab half (own bank, no false deps)
            lp = pp.tile([128, 512], F32, tag=f"lps{nh}", bufs=1,
                         name=f"lp{nh}_{p}")
            lp_pairs.setdefault(p, [None, None])[nh] = lp
            hp = hp_t[p]
            s0 = nh * VH          # vocab start in lin_W
            for k in range(NKH):
                # hp layout [128, (chunk 8, step 2, batch 64)] — chunk-major
                # so the stationary lhsT [128, 128] is contiguous
                lhsT = hp[:, 128 * k:128 * (k + 1)]
                nc.tensor.matmul(
                    lp[:, 0:VH], lhsT=lhsT,
                    rhs=linwt_sb[:, k * VOCAB + s0:k * VOCAB + s0 + VH],
                    start=(k == 0), stop=(k == NKH - 1) and not with_linb,
                    skip_group_check=True)
            if with_linb:
                # fold lin_b in as a K=1 ones-row matmul (broadcast add)
                nc.tensor.matmul(
                    lp[:, 0:VH], lhsT=onesrow_sb[:],
                    rhs=linbbf_sb[:, s0:s0 + VH],
                    start=False, stop=True, skip_group_check=True)
            if nh == 1:
                del hp_t[p]

        lp_pairs = {}   # pair -> [lp0, lp1] psum tiles awaiting copy-out

        # gate processing order: f first (it heads the serial
        # c -> tanh -> h chain), o last (needed latest by the chain)
        GORDER = [1, 2, 0, 3]   # torch gate indices f, g, i, o

        # ---------------- main loop ----------------
        for t in range(steps):
            # Pool: embedding gather two steps ahead (Pool does only DMA)
            if t + 2 < steps:
                emb_t[t + 2] = gather(t + 2)

            # PE: transposes for t+1 (DVE converts to fp8)
            if t + 1 < steps:
                ef8_t[t + 1] = transp(t + 1, emb_t.pop(t + 1))

            # per-gate psum tiles (one bank each); a gate's matmul group for
            # step t starts long after its step t-1 ACT read, so bufs=1
            # carries no WAR stall
            gt = {g: pp.tile([128, 512], F32, tag=f"gp{g}", bufs=1,
                             name=f"gp{g}_{t}") for g in GORDER}
            ef8 = ef8_t.pop(t)

            # PE: logits gap filler (one vocab half per step)
            if t >= 3:
                logits_half(t)

            # PE: gate matmul groups, gate-major, split into m-halves so
            # the ACT reads pipeline with the matmuls.  Within an m-half:
            # dk0..3 (ih + first h half) for all its m-tiles, then dk4..5.
            for g in GORDER:
                for mh in range(2):
                    for j in range(4 * mh, 4 * mh + 4):
                        m = g * 8 + j
                        reg = j * 64
                        out_ap = gt[g][:, reg:reg + 64]
                        for dk in (0, 1):
                            # exactly ONE start per psum bank per step: a
                            # start poisons the whole 2KB bank as pending-
                            # zero, which auto-zeroes every region's first
                            # write; a second start would clobber siblings
                            dr_mm(out_ap, (m * NDK + dk) * 256,
                                  ef8[:, 128 * dk:128 * (dk + 1)].rearrange(
                                      "p (two n) -> p two n", two=2),
                                  start=(mh == 0 and j == 0 and dk == 0),
                                  stop=False)
                        if with_bias:
                            nc.tensor.matmul(
                                out_ap,
                                lhsT=bb_sb[:, m * 256:m * 256 + 256]
                                .rearrange("p (two m2) -> p two m2", two=2),
                                rhs=ones_sb[:].rearrange(
                                    "p (two n) -> p two n", two=2),
                                start=False, stop=False, perf_mode=DR,
                                skip_group_check=True)
                        for dk in (2, 3):
                            dr_mm(out_ap, (m * NDK + dk) * 256,
                                  hf8_prev[:, 128 * (dk - 2):128 * (dk - 1)]
                                  .rearrange("p (two n) -> p two n", two=2),
                                  start=False, stop=False)
                    for j in range(4 * mh, 4 * mh + 4):
                        m = g * 8 + j
                        reg = j * 64
                        for dk in (4, 5):
                            dr_mm(gt[g][:, reg:reg + 64],
                                  (m * NDK + dk) * 256,
                                  hf8_prev[:, 128 * (dk - 2):128 * (dk - 1)]
                                  .rearrange("p (two n) -> p two n", two=2),
                                  start=False, stop=(dk == 5))

            # ACT: gate nonlinearities, full tiles (the serial ff->gg->ii
            # sequence gates the c chain; fewer, larger legs finish sooner)
            ff = pb.tile([128, 512], BF16, tag="ff", bufs=1, name=f"ff_{t}")
            gg = pb.tile([128, 512], BF16, tag="gg", bufs=1, name=f"gg_{t}")
            ii = pb.tile([128, 512], BF16, tag="ii", bufs=1, name=f"ii_{t}")
            oo = pb.tile([128, 512], BF16, tag="oo", bufs=1, name=f"oo_{t}")
            for dst, g, fn in ((ff, 1, AF.Sigmoid), (gg, 2, AF.Tanh),
                               (ii, 0, AF.Sigmoid), (oo, 3, AF.Sigmoid)):
                nc.scalar.activation(dst[:], gt[g][:], fn, scale=DESCALE)

            # Pool: t1 = f * c_prev (off the critical DVE/ACT engines)
            t1 = pb.tile([128, 512], BF16, tag="t1", bufs=1, name=f"t1_{t}")
            for q in range(2):
                sl = slice(256 * q, 256 * (q + 1))
                nc.gpsimd.tensor_mul(t1[:, sl], ff[:, sl], c_prev[:, sl])

            # DVE: t2 + c; ACT: tanh(c) — all in halves
            t2 = pb.tile([128, 512], BF16, tag="t2", bufs=1, name=f"t2_{t}")
            c_new = pb.tile([128, 512], BF16, tag="c", bufs=2, name=f"c_{t}")
            tc_h = pb.tile([128, 512], BF16, tag="tc", bufs=1, name=f"tc_{t}")
            for q in range(2):
                sl = slice(256 * q, 256 * (q + 1))
                nc.vector.tensor_mul(t2[:, sl], ii[:, sl], gg[:, sl])
                nc.vector.tensor_add(c_new[:, sl], t1[:, sl], t2[:, sl])
                nc.scalar.activation(tc_h[:, sl], c_new[:, sl], AF.Tanh)

            # tail in halves: t3 = o*tanh(c), hf8 (DVE); bf16 h for the
            # logits on Pool (SBUF-only operands)
            if t % 2 == 0:
                hp = pb.tile([128, 1024], BF16, tag="hp", bufs=3,
                             name=f"hp_{t // 2}")
                hp_t[t // 2] = hp
            else:
                hp = hp_t[t // 2]
            hf8 = pb.tile([128, 512], F8, tag="hf8", bufs=2, name=f"hf8_{t}")
            t3 = pb.tile([128, 512], BF16, tag="t3", bufs=1, name=f"t3_{t}")
            hp4 = hp[:].rearrange("p (cc s b) -> p cc s b", cc=8, s=2)
            for q in range(2):
                sl = slice(256 * q, 256 * (q + 1))
                nc.vector.tensor_mul(t3[:, sl], oo[:, sl], tc_h[:, sl])
                nc.vector.scalar_tensor_tensor(
                    out=hf8[:, sl], in0=t3[:, sl], scalar=SX,
                    in1=featsx_sb[:, sl], op0=ALU.mult, op1=ALU.add)
            for q in range(2):
                sl = slice(256 * q, 256 * (q + 1))
                nc.vector.tensor_add(
                    hp4[:, 4 * q:4 * (q + 1), t % 2, :],
                    t3[:, sl].rearrange("p (cc b) -> p cc b", cc=4),
                    featbf_sb[:, sl].rearrange("p (cc b) -> p cc b", cc=4))
            hf8_prev = hf8
            c_prev = c_new

        # ---------------- drain remaining logits ----------------
        for t in range(steps, steps + 4):
            if t >= 3:
                logits_half(t)

        # Deferred psum -> SBUF copies + output DMAs, emitted last so the
        # greedy scheduler places them only in genuinely idle ACT slots
        # (they must still land before the next-but-one pair reuses the
        # psum bank, which the scheduler's WAR handling enforces).
        for p in sorted(lp_pairs):
            lp0, lp1 = lp_pairs[p]
            ls = pb.tile([128, VOCAB], F32, tag="ls", bufs=2, name=f"ls_{p}")
            nc.scalar.activation(ls[:, 0:VH], lp0[:, 0:VH], AF.Copy)
            nc.scalar.activation(ls[:, VH:VOCAB], lp1[:, 0:VH], AF.Copy)
            nc.sync.dma_start(outd[:, 2 * p, :], ls[0:BL])
            nc.sync.dma_start(outd[:, 2 * p + 1, :], ls[BL:128])


# ---------------------------------------------------------------------------
# host glue
# ---------------------------------------------------------------------------

_CACHE = {}


def _get_program(steps=TS, with_bias=False, with_linb=False):
    key = (steps, with_bias, with_linb)
    if key not in _CACHE:
        _CACHE[key] = build_program(steps, with_bias, with_linb)
    return _CACHE[key]


def make_in_maps(feature, captions, embed_W, W_ih, W_hh, b_ih, b_hh,
                 lin_W, lin_b, steps=TS):
    f32 = np.float32
    bvec = (np.asarray(b_ih) + np.asarray(b_hh)).astype(f32)
    with_bias = bool(np.any(bvec != 0.0))
    with_linb = bool(np.any(np.asarray(lin_b) != 0.0))

    # stationary fp8 blob: block (m, dk, i) = W_all.T chunk
    W_all = np.concatenate([W_ih.astype(f32), W_hh.astype(f32)], axis=1)
    WT = np.ascontiguousarray(W_all.T) * SW           # [1536, 4096]
    arr = WT.reshape(NDK, 2, 128, NM, 128)            # [dk, i, p, m, ml]
    wblob = np.ascontiguousarray(
        arr.transpose(2, 3, 0, 1, 4).reshape(128, WCOLS)).astype(NPF8)

    linwt_p = np.ascontiguousarray(
        lin_W.astype(f32).T.reshape(NKH, 128, VOCAB)
        .transpose(1, 0, 2).reshape(128, NKH * VOCAB)).astype(NPBF)

    shared = {
        "wblob": wblob,
        "embbf": np.ascontiguousarray(embed_W.astype(f32)).astype(NPBF),
        "linwt": linwt_p,
        "linb": lin_b.astype(f32).reshape(1, VOCAB).astype(NPBF),
    }
    if with_bias:
        # block m: [p, i, ml]; only (p=0, i=0) row nonzero = b[gate]*SW
        bb = np.zeros((128, NM, 2, 128), dtype=f32)
        bb[0, :, 0, :] = (bvec * SW).reshape(NM, 128)
        shared["bblk"] = np.ascontiguousarray(
            bb.reshape(128, NM * 256)).astype(NPF8)

    in_maps = []
    for i in range(NCORES):
        sl = slice(i * BL, (i + 1) * BL)
        fl = feature[sl].astype(f32)                  # [64, 1024]
        featT = np.ascontiguousarray(
            fl.T.reshape(NKH, 128, BL).transpose(1, 0, 2).reshape(128, 512))
        m = dict(shared)
        m["featbf"] = featT.astype(NPBF)
        m["featsx"] = np.ascontiguousarray(featT * SX)
        m["h0f8"] = (featT * SX).astype(NPF8)
        cp = np.zeros((BL, TS), np.int32)
        cp[:, :steps] = captions[sl, :steps].astype(np.int32)
        m["caps"] = cp
        in_maps.append(m)
    return in_maps, with_bias, with_linb


def kernel(feature, captions, lengths=None, embed_W=None, W_ih=None,
           W_hh=None, b_ih=None, b_hh=None, lin_W=None, lin_b=None,
           trace=False, steps=TS):
    feature = np.asarray(feature)
    captions = np.asarray(captions)
    in_maps, with_bias, with_linb = make_in_maps(
        feature, captions, np.asarray(embed_W), np.asarray(W_ih),
        np.asarray(W_hh), np.asarray(b_ih), np.asarray(b_hh),
        np.asarray(lin_W), np.asarray(lin_b), steps=steps)
    nc = _get_program(steps, with_bias, with_linb)
    res = run_bass_kernel_spmd(nc, in_maps, list(range(NCORES)), trace=trace)
    outp = np.concatenate([res.results[i]["out"] for i in range(NCORES)],
                          axis=0)
    if trace:
        kernel.last_exec_time_ns = res.exec_time_ns
        kernel.last_results = res
    return outp


# revision 4
# speedup vs baseline: 2.9509x; 1.0152x over previous
"""Trainium2 Bass kernel v2 for the LSTM caption decoder.

Architecture (vs v1 baseline): the gate matmuls are FLIPPED — weights are the
stationary operand [K=128, M=128 gate units] and the hidden state streams as
the moving operand in transposed layout [K, batch=64].  TimelineSim charges
out_free x cycles_per_row, so the per-step gate cost drops from 32768 cycles
(h-stationary, M=64 wastes half the array) to 16384 in bf16 — and to 6144
with fp8e4 DoubleRow (2 K-tiles per instruction at 0.5 cyc/row).

Consequences of the flip:
  - gates emerge in [gate_unit, batch] = transposed layout; the whole
    elementwise c/h chain runs in [hidden, batch]; h_new IS the next step's
    moving operand. No per-step PE transposes of h.
  - the x-projection is FUSED into the gate matmul as 4 extra contraction
    chunks (W_ih columns): no token_proj phase, no [64,4096] gather; instead a
    small per-step embedding row gather [64,512] + 4 PE transposes.
  - logits keep h-stationary orientation (lhsT = h pair [128, 2x64],
    moving = lin_W.T in bf16) and run as PE gap filler, one vocab half per
    step. fp8 is NOT used for logits (accuracy).

Scaling for fp8e4 (ml_dtypes.float8_e4m3, max 240): W x 1024, x/h x 16;
psum is scaled 2^14, descaled in the ACT that applies the nonlinearity.
Numerics sim: rel err 9.0e-3 vs tolerance 2e-2.

Bias note: b_ih+b_hh from setup_inputs is always zero. If nonzero, an extra
stationary block + constant moving column adds the bias per gate unit.
"""

import sys

if "/opt/trn_rl_repo" not in sys.path:
    sys.path.insert(0, "/opt/trn_rl_repo")

import numpy as np
import ml_dtypes

import concourse.bass as bass
import concourse.mybir as mybir
import concourse.tile as tile
from concourse import bacc
from concourse.bass_utils import run_bass_kernel_spmd
from concourse.masks import make_identity

F32 = mybir.dt.float32
BF16 = mybir.dt.bfloat16
F8 = mybir.dt.float8e4
I32 = mybir.dt.int32
AF = mybir.ActivationFunctionType
ALU = mybir.AluOpType
DR = mybir.MatmulPerfMode.DoubleRow
NPF8 = ml_dtypes.float8_e4m3
NPBF = ml_dtypes.bfloat16

EMBED, HIDDEN, VOCAB = 512, 1024, 1004
B, T = 512, 65
NCORES = 8
BL = B // NCORES          # 64 batch rows per core
TS = T - 1                # 64 time steps
G4 = 4 * HIDDEN           # 4096 gate width
NM = G4 // 128            # 32 stationary m-tiles
NKI = EMBED // 128        # 4 ih k-chunks -> 2 dk pairs
NKH = HIDDEN // 128       # 8 hh k-chunks -> 4 dk pairs
NDK = (NKI + NKH) // 2    # 6 dk pairs per m-tile
VH = VOCAB // 2           # 502 vocab half

SW = 1024.0               # weight scale for fp8
SX = 16.0                 # x/h scale for fp8
DESCALE = 1.0 / (SW * SX)

# m-tile issue order: gate stream order g, i, o, f (torch gate index 2,0,3,1)
# f is streamed LAST: it heads the serial c->tanh->h chain, and putting it
# last lets the next step's ih matmuls for g/i/o start with no psum WAR wait.
GSTREAM = [2, 0, 3, 1]
MORDER = [g * 8 + j for g in GSTREAM for j in range(8)]

WCOLS = NM * NDK * 256    # 49152 fp8 cols of the stationary blob


def build_program(steps=TS, with_bias=False, with_linb=False):
    nc = bacc.Bacc("TRN2", target_bir_lowering=False, debug=False)

    wblob = nc.dram_tensor("wblob", [128, WCOLS], F8, kind="ExternalInput")
    embbf = nc.dram_tensor("embbf", [VOCAB, EMBED], BF16, kind="ExternalInput")
    linwt = nc.dram_tensor("linwt", [128, NKH * VOCAB], BF16,
                           kind="ExternalInput")
    linb = nc.dram_tensor("linb", [1, VOCAB], BF16, kind="ExternalInput")
    featbf = nc.dram_tensor("featbf", [128, 512], BF16, kind="ExternalInput")
    featsx = nc.dram_tensor("featsx", [128, 512], F32, kind="ExternalInput")
    h0f8 = nc.dram_tensor("h0f8", [128, 512], F8, kind="ExternalInput")
    caps = nc.dram_tensor("caps", [BL, TS], I32, kind="ExternalInput")
    if with_bias:
        bblk = nc.dram_tensor("bblk", [128, NM * 256], F8, kind="ExternalInput")
    else:
        bblk = None
    outd = nc.dram_tensor("out", [BL, TS, VOCAB], F32, kind="ExternalOutput")

    with tile.TileContext(nc) as tc:
        _body(nc, tc, steps, with_bias, with_linb, wblob.ap(), embbf.ap(),
              linwt.ap(), linb.ap(), featbf.ap(), featsx.ap(), h0f8.ap(),
              caps.ap(), bblk.ap() if bblk is not None else None, outd.ap())
    nc.compile()
    return nc


def _body(nc, tc, steps, with_bias, with_linb, wblob, embbf, linwt, linb,
          featbf, featsx, h0f8, caps, bblk, outd):
    with (
        tc.tile_pool(name="pg", bufs=1) as pg,
        tc.tile_pool(name="pb", bufs=1) as pb,
        tc.tile_pool(name="pp", bufs=1, space="PSUM") as pp,
    ):
        # ---------------- startup loads ----------------
        # small, early-needed tensors first: caps gates the first embedding
        # gather; feat/h0 gate the first h-dependent matmuls
        caps_sb = pg.tile([BL, TS], I32, tag="cap")
        nc.sync.dma_start(caps_sb[:], caps)
        ident = pg.tile([BL, BL], BF16, tag="id")
        make_identity(nc, ident[:])
        hf8_prev = pb.tile([128, 512], F8, tag="hf8", bufs=2, name="hf8_init")
        nc.sync.dma_start(hf8_prev[:], h0f8)
        c_prev = pb.tile([128, 512], BF16, tag="c", bufs=2, name="c_init")
        nc.vector.memset(c_prev[:], 0.0)
        featbf_sb = pg.tile([128, 512], BF16, tag="fb")
        nc.sync.dma_start(featbf_sb[:], featbf)
        featsx_sb = pg.tile([128, 512], F32, tag="fs")
        nc.sync.dma_start(featsx_sb[:], featsx)
        w_sb = pg.tile([128, WCOLS], F8, tag="w")
        # weight blocks in gate processing order (f, g, i, o) so the first
        # gate groups can start before the whole blob lands
        for g in (1, 2, 0, 3):
            c0, c1 = g * 8 * 1536, (g + 1) * 8 * 1536
            nc.sync.dma_start(w_sb[:, c0:c1], wblob[:, c0:c1])
        linwt_sb = pg.tile([128, NKH * VOCAB], BF16, tag="lw")
        nc.sync.dma_start(linwt_sb[:], linwt)
        if with_linb:
            linbbf_sb = pg.tile([1, VOCAB], BF16, tag="lb")
            nc.sync.dma_start(linbbf_sb[:], linb)
            onesrow_sb = pg.tile([1, 128], BF16, tag="ones1")
            nc.vector.memset(onesrow_sb[:], 1.0)
        if with_bias:
            bb_sb = pg.tile([128, NM * 256], F8, tag="bb")
            nc.sync.dma_start(bb_sb[:], bblk)
            ones_sb = pg.tile([128, 256], F8, tag="ones")
            nc.vector.memset(ones_sb[:], 0.0)
            nc.vector.memset(ones_sb[0:1, 0:64], SX)

        # ---------------- helpers ----------------
        def gather(t):
            e = pb.tile([BL, EMBED], BF16, tag="emb", bufs=4, name=f"emb_{t}")
            nc.gpsimd.indirect_dma_start(
                out=e[:], out_offset=None, in_=embbf,
                in_offset=bass.IndirectOffsetOnAxis(
                    ap=caps_sb[:, t:t + 1], axis=0))
            return e

        def transp(t, emb_t):
            ps = pp.tile([128, 256], BF16, tag="et", bufs=2, name=f"etp_{t}")
            for cchunk in range(4):
                nc.tensor.matmul(
                    ps[:, 64 * cchunk:64 * (cchunk + 1)],
                    lhsT=emb_t[:, 128 * cchunk:128 * (cchunk + 1)],
                    rhs=ident[:], is_transpose=True)
            ef = pb.tile([128, 256], F8, tag="ef8", bufs=2, name=f"ef8_{t}")
            nc.vector.tensor_scalar_mul(ef[:], ps[:], SX)
            return ef

        def dr_mm(out_ap, wcol, rhs_ap, start, stop):
            nc.tensor.matmul(
                out_ap,
                lhsT=w_sb[:, wcol:wcol + 256].rearrange(
                    "p (two m) -> p two m", two=2),
                rhs=rhs_ap, start=start, stop=stop, perf_mode=DR,
                skip_group_check=True)

        # pre-loop: embeddings for steps 0 and 1
        emb_t = {0: gather(0)}
        if steps > 1:
            emb_t[1] = gather(1)
        ef8_t = {0: transp(0, emb_t[0])}

        hp_t = {}         # hpair tiles by pair index

        def logits_half(t):
            # pair p halves run at iterations 2p+3 / 2p+4 — both at least
            # two steps after hp(p) completes, so the PE never waits on it
            p, nh = (t - 3) // 2, (t - 3) % 2
            if p > (steps - 2) // 2:
                return
            # separate psum tile per vocab half (own bank, no false deps)
            lp = pp.tile([128, 512], F32, tag=f"lps{nh}", bufs=1,
                         name=f"lp{nh}_{p}")
            lp_pairs.setdefault(p, [None, None])[nh] = lp
            hp = hp_t[p]
            s0 = nh * VH          # vocab start in lin_W
            for k in range(NKH):
                # hp layout [128, (chunk 8, step 2, batch 64)] — chunk-major
                # so the stationary lhsT [128, 128] is contiguous
                lhsT = hp[:, 128 * k:128 * (k + 1)]
                nc.tensor.matmul(
                    lp[:, 0:VH], lhsT=lhsT,
                    rhs=linwt_sb[:, k * VOCAB + s0:k * VOCAB + s0 + VH],
                    start=(k == 0), stop=(k == NKH - 1) and not with_linb,
                    skip_group_check=True)
            if with_linb:
                # fold lin_b in as a K=1 ones-row matmul (broadcast add)
                nc.tensor.matmul(
                    lp[:, 0:VH], lhsT=onesrow_sb[:],
                    rhs=linbbf_sb[:, s0:s0 + VH],
                    start=False, stop=True, skip_group_check=True)
            if nh == 1:
                del hp_t[p]

        lp_pairs = {}   # pair -> [lp0, lp1] psum tiles awaiting copy-out

        # gate processing order: f first (it heads the serial
        # c -> tanh -> h chain), o last (needed latest by the chain)
        GORDER = [1, 2, 0, 3]   # torch gate indices f, g, i, o

        # ---------------- main loop ----------------
        for t in range(steps):
            # Pool: embedding gather two steps ahead (Pool does only DMA)
            if t + 2 < steps:
                emb_t[t + 2] = gather(t + 2)

            # PE: transposes for t+1 (DVE converts to fp8)
            if t + 1 < steps:
                ef8_t[t + 1] = transp(t + 1, emb_t.pop(t + 1))

            # per-gate psum tiles (one bank each); a gate's matmul group for
            # step t starts long after its step t-1 ACT read, so bufs=1
            # carries no WAR stall
            gt = {g: pp.tile([128, 512], F32, tag=f"gp{g}", bufs=1,
                             name=f"gp{g}_{t}") for g in GORDER}
            ef8 = ef8_t.pop(t)

            # PE: logits gap filler (one vocab half per step)
            if t >= 3:
                logits_half(t)

            # PE: gate matmul groups, gate-major, split into m-halves so
            # the ACT reads pipeline with the matmuls.  Within an m-half:
            # dk0..3 (ih + first h half) for all its m-tiles, then dk4..5.
            for g in GORDER:
                for mh in range(2):
                    for j in range(4 * mh, 4 * mh + 4):
                        m = g * 8 + j
                        reg = j * 64
                        out_ap = gt[g][:, reg:reg + 64]
                        for dk in (0, 1):
                            # exactly ONE start per psum bank per step: a
                            # start poisons the whole 2KB bank as pending-
                            # zero, which auto-zeroes every region's first
                            # write; a second start would clobber siblings
                            dr_mm(out_ap, (m * NDK + dk) * 256,
                                  ef8[:, 128 * dk:128 * (dk + 1)].rearrange(
                                      "p (two n) -> p two n", two=2),
                                  start=(mh == 0 and j == 0 and dk == 0),
                                  stop=False)
                        if with_bias:
                            nc.tensor.matmul(
                                out_ap,
                                lhsT=bb_sb[:, m * 256:m * 256 + 256]
                                .rearrange("p (two m2) -> p two m2", two=2),
                                rhs=ones_sb[:].rearrange(
                                    "p (two n) -> p two n", two=2),
                                start=False, stop=False, perf_mode=DR,
                                skip_group_check=True)
                        for dk in (2, 3):
                            dr_mm(out_ap, (m * NDK + dk) * 256,
                                  hf8_prev[:, 128 * (dk - 2):128 * (dk - 1)]
                                  .rearrange("p (two n) -> p two n", two=2),
                                  start=False, stop=False)
                    for j in range(4 * mh, 4 * mh + 4):
                        m = g * 8 + j
                        reg = j * 64
                        for dk in (4, 5):
                            dr_mm(gt[g][:, reg:reg + 64],
                                  (m * NDK + dk) * 256,
                                  hf8_prev[:, 128 * (dk - 2):128 * (dk - 1)]
                                  .rearrange("p (two n) -> p two n", two=2),
                                  start=False, stop=(dk == 5))

            # ACT: gate nonlinearities, full tiles (the serial ff->gg->ii
            # sequence gates the c chain; fewer, larger legs finish sooner)
            ff = pb.tile([128, 512], BF16, tag="ff", bufs=1, name=f"ff_{t}")
            gg = pb.tile([128, 512], BF16, tag="gg", bufs=1, name=f"gg_{t}")
            ii = pb.tile([128, 512], BF16, tag="ii", bufs=1, name=f"ii_{t}")
            oo = pb.tile([128, 512], BF16, tag="oo", bufs=1, name=f"oo_{t}")
            for dst, g, fn in ((ff, 1, AF.Sigmoid), (gg, 2, AF.Tanh),
                               (ii, 0, AF.Sigmoid), (oo, 3, AF.Sigmoid)):
                nc.scalar.activation(dst[:], gt[g][:], fn, scale=DESCALE)

            # Pool: t1 = f * c_prev (off the critical DVE/ACT engines)
            t1 = pb.tile([128, 512], BF16, tag="t1", bufs=1, name=f"t1_{t}")
            for q in range(2):
                sl = slice(256 * q, 256 * (q + 1))
                nc.gpsimd.tensor_mul(t1[:, sl], ff[:, sl], c_prev[:, sl])

            # DVE: t2 + c; ACT: tanh(c) — all in halves
            t2 = pb.tile([128, 512], BF16, tag="t2", bufs=1, name=f"t2_{t}")
            c_new = pb.tile([128, 512], BF16, tag="c", bufs=2, name=f"c_{t}")
            tc_h = pb.tile([128, 512], BF16, tag="tc", bufs=1, name=f"tc_{t}")
            for q in range(2):
                sl = slice(256 * q, 256 * (q + 1))
                nc.vector.tensor_mul(t2[:, sl], ii[:, sl], gg[:, sl])
                nc.vector.tensor_add(c_new[:, sl], t1[:, sl], t2[:, sl])
                nc.scalar.activation(tc_h[:, sl], c_new[:, sl], AF.Tanh)

            # tail in halves: t3 = o*tanh(c), hf8 (DVE); bf16 h for the
            # logits on Pool (SBUF-only operands)
            if t % 2 == 0:
                hp = pb.tile([128, 1024], BF16, tag="hp", bufs=3,
                             name=f"hp_{t // 2}")
                hp_t[t // 2] = hp
            else:
                hp = hp_t[t // 2]
            hf8 = pb.tile([128, 512], F8, tag="hf8", bufs=2, name=f"hf8_{t}")
            t3 = pb.tile([128, 512], BF16, tag="t3", bufs=1, name=f"t3_{t}")
            hp4 = hp[:].rearrange("p (cc s b) -> p cc s b", cc=8, s=2)
            for q in range(2):
                sl = slice(256 * q, 256 * (q + 1))
                nc.vector.tensor_mul(t3[:, sl], oo[:, sl], tc_h[:, sl])
                nc.vector.scalar_tensor_tensor(
                    out=hf8[:, sl], in0=t3[:, sl], scalar=SX,
                    in1=featsx_sb[:, sl], op0=ALU.mult, op1=ALU.add)
            for q in range(2):
                sl = slice(256 * q, 256 * (q + 1))
                nc.vector.tensor_add(
                    hp4[:, 4 * q:4 * (q + 1), t % 2, :],
                    t3[:, sl].rearrange("p (cc b) -> p cc b", cc=4),
                    featbf_sb[:, sl].rearrange("p (cc b) -> p cc b", cc=4))
            hf8_prev = hf8
            c_prev = c_new

        # ---------------- drain remaining logits ----------------
        for t in range(steps, steps + 4):
            if t >= 3:
                logits_half(t)

        # Deferred psum -> SBUF copies + output DMAs, emitted last so the
        # greedy scheduler places them only in genuinely idle ACT slots
        # (they must still land before the next-but-one pair reuses the
        # psum bank, which the scheduler's WAR handling enforces).
        for p in sorted(lp_pairs):
            lp0, lp1 = lp_pairs[p]
            ls = pb.tile([128, VOCAB], F32, tag="ls", bufs=2, name=f"ls_{p}")
            # quarter-sized copies cap how long a copy can occupy ACT when
            # the scheduler slots one just before a chain leg becomes ready
            for q in range(2):
                qs = slice(q * 251, (q + 1) * 251)
                nc.scalar.activation(ls[:, q * 251:(q + 1) * 251],
                                     lp0[:, qs], AF.Copy)
            for q in range(2):
                nc.scalar.activation(
                    ls[:, VH + q * 251:VH + (q + 1) * 251],
                    lp1[:, q * 251:(q + 1) * 251], AF.Copy)
            nc.sync.dma_start(outd[:, 2 * p, :], ls[0:BL])
            nc.sync.dma_start(outd[:, 2 * p + 1, :], ls[BL:128])


# ---------------------------------------------------------------------------
# host glue
# ---------------------------------------------------------------------------

_CACHE = {}


def _get_program(steps=TS, with_bias=False, with_linb=False):
    key = (steps, with_bias, with_linb)
    if key not in _CACHE:
        _CACHE[key] = build_program(steps, with_bias, with_linb)
    return _CACHE[key]


def make_in_maps(feature, captions, embed_W, W_ih, W_hh, b_ih, b_hh,
                 lin_W, lin_b, steps=TS):
    f32 = np.float32
    bvec = (np.asarray(b_ih) + np.asarray(b_hh)).astype(f32)
    with_bias = bool(np.any(bvec != 0.0))
    with_linb = bool(np.any(np.asarray(lin_b) != 0.0))

    # stationary fp8 blob: block (m, dk, i) = W_all.T chunk
    W_all = np.concatenate([W_ih.astype(f32), W_hh.astype(f32)], axis=1)
    WT = np.ascontiguousarray(W_all.T) * SW           # [1536, 4096]
    arr = WT.reshape(NDK, 2, 128, NM, 128)            # [dk, i, p, m, ml]
    wblob = np.ascontiguousarray(
        arr.transpose(2, 3, 0, 1, 4).reshape(128, WCOLS)).astype(NPF8)

    linwt_p = np.ascontiguousarray(
        lin_W.astype(f32).T.reshape(NKH, 128, VOCAB)
        .transpose(1, 0, 2).reshape(128, NKH * VOCAB)).astype(NPBF)

    shared = {
        "wblob": wblob,
        "embbf": np.ascontiguousarray(embed_W.astype(f32)).astype(NPBF),
        "linwt": linwt_p,
        "linb": lin_b.astype(f32).reshape(1, VOCAB).astype(NPBF),
    }
    if with_bias:
        # block m: [p, i, ml]; only (p=0, i=0) row nonzero = b[gate]*SW
        bb = np.zeros((128, NM, 2, 128), dtype=f32)
        bb[0, :, 0, :] = (bvec * SW).reshape(NM, 128)
        shared["bblk"] = np.ascontiguousarray(
            bb.reshape(128, NM * 256)).astype(NPF8)

    in_maps = []
    for i in range(NCORES):
        sl = slice(i * BL, (i + 1) * BL)
        fl = feature[sl].astype(f32)                  # [64, 1024]
        featT = np.ascontiguousarray(
            fl.T.reshape(NKH, 128, BL).transpose(1, 0, 2).reshape(128, 512))
        m = dict(shared)
        m["featbf"] = featT.astype(NPBF)
        m["featsx"] = np.ascontiguousarray(featT * SX)
        m["h0f8"] = (featT * SX).astype(NPF8)
        cp = np.zeros((BL, TS), np.int32)
        cp[:, :steps] = captions[sl, :steps].astype(np.int32)
        m["caps"] = cp
        in_maps.append(m)
    return in_maps, with_bias, with_linb


def kernel(feature, captions, lengths=None, embed_W=None, W_ih=None,
           W_hh=None, b_ih=None, b_hh=None, lin_W=None, lin_b=None,
           trace=False, steps=TS):
    feature = np.asarray(feature)
    captions = np.asarray(captions)
    in_maps, with_bias, with_linb = make_in_maps(
        feature, captions, np.asarray(embed_W), np.asarray(W_ih),
        np.asarray(W_hh), np.asarray(b_ih), np.asarray(b_hh),
        np.asarray(lin_W), np.asarray(lin_b), steps=steps)
    nc = _get_program(steps, with_bias, with_linb)
    res = run_bass_kernel_spmd(nc, in_maps, list(range(NCORES)), trace=trace)
    outp = np.concatenate([res.results[i]["out"] for i in range(NCORES)],
                          axis=0)
    if trace:
        kernel.last_exec_time_ns = res.exec_time_ns
        kernel.last_results = res
    return outp
